# revision 5
# baseline (speedup 1.0000x reference)
"""Single-head attention (B=4, Lq=Lkv=4096, D=128) on 8 TRN2 NeuronCores.

Sharding: data-parallel over (batch, query-half). Core c handles batch c//2,
query rows (c%2)*2048 ... +2048, with full K/V for that batch. No collectives.

v2 design (vs the PE-transpose baseline):
  - The host pre-transposes x1/x2/x3 to [d, L] and passes W^T directly, so the
    kernel does no PE transposes and no PSUM->SBUF staging copies; inputs DMA
    straight into SBUF as projection operands.
  - Wq/bq are pre-scaled by A = 1024*log2(e)/sqrt(d) so scores arrive in the
    fp16-bits log domain: the ACT exp folds the un-scale into its scale arg,
    and 8 of 64 exp tiles run on DVE as a one-instruction Schraudolph
    (bits = round(st + B) -> uint16, bitcast fp16), freeing ACT cycles.
    All E tiles carry a global 2^-3 factor (cancels in softmax).
  - Softmax denominator partial sums: pair-adds on Pool(GpSimd), chain on DVE,
    all fp16 (DVE 2x mode).
  - qh-outer / kt-inner loop; K/V projections for later groups are interleaved
    into the first iterations so the PE never idles on input DMA; qh0's
    epilogue is issued after qh1's first two iterations so the sum-chain tail
    overlaps PE work.
  - O^T accumulates in PSUM over 32 k-tiles (fp16 V x fp16 E matmuls);
    epilogue: ones-matmul partition-reduce of the E sum, DVE reciprocal +
    multiply, chunked DMA out of O^T. Host transposes O^T when stitching.

End-to-end scale-relative absmax error vs fp32 reference: ~6e-3 (the
Schraudolph tiles dominate; pure-ACT config measures ~3e-4).
"""

import os
import sys

os.environ.setdefault("NEURON_RT_RESET_CORES", "1")

if "/opt/trn_rl_repo" not in sys.path:
    sys.path.insert(0, "/opt/trn_rl_repo")

from contextlib import ExitStack

import numpy as np

import concourse.bass as bass  # noqa: F401
import concourse.bacc as bacc
import concourse.tile as tile
from concourse import mybir
from concourse._compat import with_exitstack
from concourse.bass_utils import run_bass_kernel_spmd

D = 128
LQ = 2048  # per-core query slab
LKV = 4096
QH = 1024  # query chunk per pass (2 passes)
NKT = LKV // 128  # 32
NKG = LKV // 1024  # 4 k/v groups
SCALE = float(1.0 / np.sqrt(128.0))
A_PRE = float(1024.0 * np.log2(np.e) * SCALE)  # folded into Wq/bq host-side
ACT_SCALE = float(1.0 / (1024.0 * np.log2(np.e)))  # st*ACT_SCALE = raw*SCALE
PSI = 3  # all E tiles scaled by 2^-PSI (cancels in softmax)
ACT_BIAS = float(-PSI * np.log(2.0))
B_SCH = 12243.25 - 1024.0 * PSI + 12288.0 - 12288.0  # fit for PSI=3: see numerics2
B_SCH = 12243.25  # round(st + B) -> fp16 bits of exp(raw*SCALE)*2^-3
SCH_KT = (2, 10, 18, 26)  # per-qh k-tiles whose exp runs on DVE (8 of 64 total)

F32 = mybir.dt.float32
F32R = mybir.dt.float32r
FP16 = mybir.dt.float16
U16 = mybir.dt.uint16


@with_exitstack
def attn_body(ctx: ExitStack, tc: tile.TileContext, io: dict):
    nc = tc.nc
    ctx.enter_context(
        nc.allow_low_precision(
            reason="f32r operands for full-rate PE matmul; fp16 P/V; fp32 PSUM"
        )
    )
    x1, x2, x3 = io["x1"], io["x2"], io["x3"]
    out = io["o"]

    # Packed constants: cols 0:128 WqT*A | 128:256 WkT | 256:384 WvT |
    # 384 bq*A | 385 bk | 386 bv.   (W^T = [d_in, e_out], host-transposed.)
    consts = ctx.enter_context(tc.tile_pool(name="consts", bufs=1))
    wpk = consts.tile([128, 384], F32R)
    nc.sync.dma_start(out=wpk, in_=io["wpack"][:, 0:384].bitcast(F32R))
    bvec = consts.tile([128, 4], F32)
    nc.sync.dma_start(out=bvec, in_=io["wpack"][:, 384:388])
    wT = {"q": wpk[:, 0:128], "k": wpk[:, 128:256], "v": wpk[:, 256:384]}
    bias = {"q": bvec[:, 0:1], "k": bvec[:, 1:2], "v": bvec[:, 2:3]}
    actbias = bvec[:, 3:4]
    ones_mat = consts.tile([128, 128], FP16)
    nc.vector.memset(ones_mat, 1.0)

    # Persistent activations.
    acts = ctx.enter_context(tc.tile_pool(name="acts", bufs=1))
    qt_g = [acts.tile([128, QH], F32R, tag=f"qt{g}", name=f"qt{g}") for g in range(2)]
    kt_g = [acts.tile([128, 1024], F32R, tag=f"kt{g}", name=f"kt{g}") for g in range(NKG)]
    vn_g = [acts.tile([128, 8, 128], FP16, tag=f"vn{g}", name=f"vn{g}") for g in range(NKG)]
    otn = [acts.tile([128, QH], F32, tag=f"otn{h}", name=f"otn{h}") for h in range(2)]

    def kt_tile(kt):
        return kt_g[kt // 8][:, (kt % 8) * 128 : (kt % 8 + 1) * 128]

    xst = ctx.enter_context(tc.tile_pool(name="xst", bufs=4))
    vtp = ctx.enter_context(tc.tile_pool(name="vtp", bufs=2))
    stp = ctx.enter_context(tc.tile_pool(name="stp", bufs=2, space="PSUM"))
    otp = ctx.enter_context(tc.tile_pool(name="otp", bufs=2, space="PSUM"))
    etp = ctx.enter_context(tc.tile_pool(name="etp", bufs=6))
    etu = ctx.enter_context(tc.tile_pool(name="etu", bufs=2))
    sumt = ctx.enter_context(tc.tile_pool(name="sumt", bufs=6))
    nrm = ctx.enter_context(tc.tile_pool(name="nrm", bufs=2))

    def load_group(xin, g, eng):
        raw = xst.tile([128, 1024], F32R, tag="xraw")
        eng.dma_start(out=raw, in_=xin[:, g * 1024 : (g + 1) * 1024].bitcast(F32R))
        return raw

    def project(dst, w, src, b, bias_eng):
        ps = stp.tile([128, 1024], F32, tag="ps")
        for c in range(2):
            sl = slice(c * 512, (c + 1) * 512)
            nc.tensor.matmul(ps[:, sl], w, src[:, sl], start=True, stop=True)
        bias_eng.tensor_scalar_add(out=dst, in0=ps, scalar1=b)

    def do_q(g):
        project(qt_g[g], wT["q"], load_group(x1, g, nc.sync), bias["q"], nc.vector)

    def do_k(g):
        project(kt_g[g], wT["k"], load_group(x2, g, nc.sync), bias["k"], nc.vector)

    def do_v(g):
        vt = vtp.tile([128, 1024], FP16, tag="vt")
        project(vt, wT["v"], load_group(x3, g, nc.gpsimd), bias["v"], nc.vector)
        nc.scalar.dma_start_transpose(out=vn_g[g], in_=vt)

    # Phase 1 head: only what the first iterations need; the rest interleaves
    # into the qh0 loop below.
    do_q(0)
    do_k(0)
    do_v(0)
    deferred = [lambda: do_k(1), lambda: do_v(1), lambda: do_k(2), lambda: do_v(2),
                lambda: do_k(3), lambda: do_v(3), lambda: do_q(1)]
    defer_at = {1: 0, 2: 1, 3: 2, 4: 3, 5: 4, 6: 5, 7: 6}  # qh0 kt -> deferred idx

    # ---- main loop ----
    ot = [otp.tile([128, QH], F32, tag="ot", name=f"ot{h}") for h in range(2)]
    pendings = [dict() for _ in range(2)]
    chains = [None, None]

    def sum_insert(qh, tile_, chain_eng):
        pending = pendings[qh]
        if 0 not in pending:
            pending[0] = tile_
            return
        prev = pending.pop(0)
        pair = sumt.tile([128, QH], FP16, tag="sum0", name="s0")
        nc.gpsimd.tensor_tensor(out=pair, in0=prev, in1=tile_, op=mybir.AluOpType.add)
        if chains[qh] is None:
            chains[qh] = pair
        else:
            acc = sumt.tile([128, QH], FP16, tag="sumc", name="sc")
            chain_eng.tensor_tensor(out=acc, in0=chains[qh], in1=pair, op=mybir.AluOpType.add)
            chains[qh] = acc

    # software-pipelined: O(kt) issues after S(kt+1)/exp(kt+1)
    pend_o = [None, None]  # per qh: (kt, et)

    def flush_o(qh):
        if pend_o[qh] is None:
            return
        kt, et = pend_o[qh]
        for c in range(2):
            sl = slice(c * 512, (c + 1) * 512)
            nc.tensor.matmul(
                ot[qh][:, sl],
                vn_g[kt // 8][:, kt % 8, :],
                et[:, sl],
                start=kt == 0,
                stop=kt == NKT - 1,
            )
        pend_o[qh] = None

    def iteration(kt, qh):
        st = stp.tile([128, QH], F32, tag="ps", name="st")
        for c in range(2):
            sl = slice(c * 512, (c + 1) * 512)
            nc.tensor.matmul(st[:, sl], kt_tile(kt), qt_g[qh][:, sl], start=True, stop=True)
        if kt in SCH_KT:
            eu = etu.tile([128, QH], U16, tag="etu", name="eu")
            nc.vector.tensor_scalar_add(out=eu, in0=st, scalar1=B_SCH)
            et = eu.bitcast(FP16)
            sum_insert(qh, et, nc.gpsimd)
        else:
            et = etp.tile([128, QH], FP16, tag="et", name="et")
            nc.scalar.activation(
                out=et, in_=st, func=mybir.ActivationFunctionType.Exp,
                scale=ACT_SCALE, bias=actbias,
            )
            sum_insert(qh, et, nc.vector)
        flush_o(qh)
        pend_o[qh] = (kt, et)

    def epilogue(qh):
        q0 = qh * QH
        esum = chains[qh]
        se = stp.tile([128, QH], F32, tag="ps", name="se")
        for c in range(2):
            sl = slice(c * 512, (c + 1) * 512)
            nc.tensor.matmul(se[:, sl], ones_mat, esum[:, sl], start=True, stop=True)
            rec = nrm.tile([128, 512], F32, tag="rec", name="rec")
            nc.vector.reciprocal(out=rec, in_=se[:, sl])
            nc.vector.tensor_mul(out=otn[qh][:, sl], in0=ot[qh][:, sl], in1=rec)
            nc.sync.dma_start(
                out=out[:, q0 + c * 512 : q0 + (c + 1) * 512], in_=otn[qh][:, sl]
            )

    for kt in range(NKT):
        iteration(kt, 0)
        if kt in defer_at:
            deferred[defer_at[kt]]()
    flush_o(0)
    iteration(0, 1)
    iteration(1, 1)
    epilogue(0)
    for kt in range(2, NKT):
        iteration(kt, 1)
    flush_o(1)
    epilogue(1)


def build_nc() -> "bacc.Bacc":
    nc = bacc.Bacc("TRN2", target_bir_lowering=False, debug=False, num_devices=8)
    io = {}
    io["x1"] = nc.dram_tensor("x1", [D, LQ], F32, kind="ExternalInput").ap()
    io["x2"] = nc.dram_tensor("x2", [D, LKV], F32, kind="ExternalInput").ap()
    io["x3"] = nc.dram_tensor("x3", [D, LKV], F32, kind="ExternalInput").ap()
    io["wpack"] = nc.dram_tensor("wpack", [128, 388], F32, kind="ExternalInput").ap()
    io["o"] = nc.dram_tensor("o", [128, LQ], F32, kind="ExternalOutput").ap()
    with tile.TileContext(nc) as tc:
        attn_body(tc, io)
    nc.compile()
    return nc


def make_in_maps(inputs: dict) -> list[dict]:
    wq = np.asarray(inputs["Wq"], np.float32).T * np.float32(A_PRE)
    wk = np.asarray(inputs["Wk"], np.float32).T
    wv = np.asarray(inputs["Wv"], np.float32).T
    bq = np.asarray(inputs["bq"], np.float32) * np.float32(A_PRE)
    bk = np.asarray(inputs["bk"], np.float32)
    bv = np.asarray(inputs["bv"], np.float32)
    wpack = np.concatenate(
        [wq, wk, wv, bq[:, None], bk[:, None], bv[:, None],
         np.full((128, 1), ACT_BIAS, np.float32)], axis=1
    )
    shared = {"wpack": np.ascontiguousarray(wpack)}
    x1 = np.asarray(inputs["x1"], np.float32)
    x2 = np.asarray(inputs["x2"], np.float32)
    x3 = np.asarray(inputs["x3"], np.float32)
    in_maps = []
    for c in range(8):
        b, qh = c // 2, c % 2
        in_maps.append(
            {
                "x1": np.ascontiguousarray(x1[b, qh * LQ : (qh + 1) * LQ, :].T),
                "x2": np.ascontiguousarray(x2[b].T),
                "x3": np.ascontiguousarray(x3[b].T),
                **shared,
            }
        )
    return in_maps


_NC_CACHE = None


def get_nc():
    global _NC_CACHE
    if _NC_CACHE is None:
        _NC_CACHE = build_nc()
    return _NC_CACHE


def kernel(**inputs) -> np.ndarray:
    nc = get_nc()
    in_maps = make_in_maps(inputs)
    res = run_bass_kernel_spmd(nc, in_maps, core_ids=list(range(8)))
    out = np.empty((4, 4096, 128), np.float32)
    for c in range(8):
        b, qh = c // 2, c % 2
        out[b, qh * LQ : (qh + 1) * LQ, :] = res.results[c]["o"].T
    return out


if __name__ == "__main__":
    nc = build_nc()
    print("built OK")


# revision 21
# speedup vs baseline: 1.0532x; 1.0532x over previous
"""Single-head attention (B=4, Lq=Lkv=4096, D=128) on 8 TRN2 NeuronCores.

Sharding: data-parallel over (batch, query-half). Core c handles batch c//2,
query rows (c%2)*2048 ... +2048, with full K/V for that batch. No collectives.

Per-core kernel (all engines overlapped; ACT-exp is the steady-state rate):
  - Inputs stream in 512KB groups; PE-transposes x tiles (fp32 exact) with
    batched PSUM->SBUF copies split across DVE/ACT; projections in float32r
    (full PE rate, ~19-bit); V is projected to fp16 and DMA-transposed to
    natural [k, e] layout.
  - Main loop per (k-tile, q-half): S^T = K^T_tile.T @ Q^T (float32r) into
    PSUM; ACT computes exp (scale folded) -> E^T fp16 in SBUF; DVE
    accumulates E^T partial sums (fp16 2x mode, pair+chain); PE accumulates
    O^T += V_tile.T @ E^T in PSUM.
  - Epilogue per q-half: sumexp = all-ones matmul of the E^T sum (result
    replicated across partitions), reciprocal + multiply on DVE, chunked
    DMA out of O^T. The host transposes O^T -> O when stitching.

Numerics: scores in float32r (measured indistinguishable from fp32 here);
softmax without max-subtraction (|scores/sqrt(d)| < ~8, exp is safe in
fp32/fp16); P and V in fp16. End-to-end scale-relative absmax error vs the
fp32 reference: ~4e-4 (CoreSim and hardware).
"""

import os
import sys

# Recovers wedged NeuronCores (NRT_EXEC_UNIT_UNRECOVERABLE) at init; must be
# set before the first device use.
os.environ.setdefault("NEURON_RT_RESET_CORES", "1")

if "/opt/trn_rl_repo" not in sys.path:
    sys.path.insert(0, "/opt/trn_rl_repo")

from contextlib import ExitStack

import numpy as np

import concourse.bass as bass  # noqa: F401  (bass types used via bacc/tile)
import concourse.bacc as bacc
import concourse.tile as tile
from concourse import mybir
from concourse._compat import with_exitstack
from concourse.bass_utils import run_bass_kernel_spmd

D = 128
LQ = 2048  # per-core query slab
LKV = 4096
NQT = LQ // 128  # 16
NKT = LKV // 128  # 32
QH = 1024  # q chunk processed per pass (2 passes)
NCH = QH // 512  # 512-wide matmul chunks per pass
SCALE = float(1.0 / np.sqrt(128.0))

F32 = mybir.dt.float32
F32R = mybir.dt.float32r
BF16 = mybir.dt.bfloat16
FP16 = mybir.dt.float16


@with_exitstack
def attn_body(ctx: ExitStack, tc: tile.TileContext, io: dict):
    nc = tc.nc
    ctx.enter_context(
        nc.allow_low_precision(
            reason="f32r (19-bit) operands for full-rate PE matmul; fp32 PSUM accum"
        )
    )
    x1, x2, x3 = io["x1"], io["x2"], io["x3"]
    out = io["o"]

    # All constants arrive in one packed [128, 515] tensor (one DMA, issued
    # first): cols 0:128 Wq | 128:256 Wk | 256:384 Wv | 384:512 ident |
    # 512 bq | 513 bk | 514 bv.
    consts = ctx.enter_context(tc.tile_pool(name="consts", bufs=1))
    wpk = consts.tile([128, 515], F32)
    nc.sync.dma_start(out=wpk, in_=io["wpack"])
    w_nat = {"Wq": wpk[:, 0:128], "Wk": wpk[:, 128:256], "Wv": wpk[:, 256:384]}
    ident = wpk[:, 384:512]
    bias_t = {"Wq": wpk[:, 512:513], "Wk": wpk[:, 513:514], "Wv": wpk[:, 514:515]}
    ones_mat = consts.tile([128, 128], FP16)
    nc.vector.memset(ones_mat, 1.0)

    # ---- Phase 1: weight transposes, x transposes, projections ----
    # Persistent activations for the main loop. Quartered so Tile's
    # tile-granular dependency tracking lets the main loop start as soon as
    # the first quarter of K^T exists.
    acts = ctx.enter_context(tc.tile_pool(name="acts", bufs=1))
    qt_q = [acts.tile([128, QH], F32R, tag=f"qt{i}", name=f"qt{i}") for i in range(LQ // QH)]
    kt_q = [acts.tile([128, 1024], F32R, tag=f"kt{i}", name=f"kt{i}") for i in range(LKV // 1024)]
    vn_q = [
        acts.tile([128, 8, 128], FP16, tag=f"vn{i}", name=f"vn{i}") for i in range(NKT // 8)
    ]  # V natural [k%128, kt, e], quartered

    def kt_tile(kt):  # K^T 128-col block for k-tile kt
        return kt_q[kt // 8][:, (kt % 8) * 128 : (kt % 8 + 1) * 128]

    with (
        tc.tile_pool(name="wts", bufs=1) as wts,
        tc.tile_pool(name="xraw", bufs=2) as xraw,
        tc.tile_pool(name="xT", bufs=2) as xT,
        tc.tile_pool(name="ptr", bufs=2, space="PSUM") as ptr,
        tc.tile_pool(name="pmm", bufs=2, space="PSUM") as pmm,
        tc.tile_pool(name="vtmp", bufs=2) as vtmp,
    ):
        # Weights: PE-transpose the packed naturals to W^T [d, e].
        w_T = {}
        for name in ("Wq", "Wk", "Wv"):
            pt = ptr.tile([128, 128], F32, tag="ptrans")
            nc.tensor.transpose(pt, w_nat[name], ident)
            wt = wts.tile([128, 128], F32R, tag=f"wT_{name}")
            nc.vector.tensor_copy(out=wt, in_=pt)
            w_T[name] = wt

        # Each group = 8 x-tiles = 1024 columns: own DMA load, 8 PE
        # transposes into one [128, 1024] PSUM tile, one batched copy to
        # SBUF, projection matmuls + bias-add, all group-granular so the
        # main loop can start as soon as the first K^T quarter is ready.
        GRP = 8  # group size locked by kt quarter width

        def load_group(xin, name, g, on_act):
            raw = xraw.tile([128, GRP, 128], F32, tag=f"raw_{name}")
            nc.sync.dma_start(
                out=raw,
                in_=xin.rearrange("(t p) d -> p t d", p=128)[
                    :, g * GRP : (g + 1) * GRP, :
                ],
            )
            pt = ptr.tile([128, GRP * 128], F32, tag="ptrans")
            for j in range(GRP):
                nc.tensor.transpose(pt[:, j * 128 : (j + 1) * 128], raw[:, j, :], ident)
            xt_ = xT.tile([128, GRP * 128], F32R, tag=f"xT_{name}")
            if on_act:
                nc.scalar.copy(out=xt_, in_=pt)
            else:
                nc.vector.tensor_copy(out=xt_, in_=pt)
            return xt_

        def project_group(dst, wT, src, bias):
            ps = pmm.tile([128, 1024], F32, tag="proj")
            for h in range(2):
                nc.tensor.matmul(
                    ps[:, h * 512 : (h + 1) * 512],
                    wT,
                    src[:, h * 512 : (h + 1) * 512],
                    start=True,
                    stop=True,
                )
            nc.vector.tensor_scalar_add(out=dst, in0=ps, scalar1=bias)

        # Interleave groups so the main-loop critical path (Q^T half 0 and
        # K^T quarter 0, then V quarter 0) is produced first.
        work = [("x1", 0), ("x2", 0), ("x3", 0), ("x1", 1), ("x2", 1), ("x3", 1),
                ("x2", 2), ("x3", 2), ("x2", 3), ("x3", 3)]
        for name, g in work:
            if name == "x1":
                src = load_group(x1, "x1", g, on_act=False)
                project_group(qt_q[g], w_T["Wq"], src, bias_t["Wq"])
            elif name == "x2":
                src = load_group(x2, "x2", g, on_act=True)
                project_group(kt_q[g], w_T["Wk"], src, bias_t["Wk"])
            else:
                src = load_group(x3, "x3", g, on_act=True)
                vt = vtmp.tile([128, GRP * 128], FP16, tag="vT")
                project_group(vt, w_T["Wv"], src, bias_t["Wv"])
                nc.scalar.dma_start_transpose(out=vn_q[g], in_=vt)

    # ---- Phase 2: attention main loop ----
    otn_h = [acts.tile([128, QH], F32, tag=f"otn{i}", name=f"otn{i}") for i in range(LQ // QH)]
    with (
        tc.tile_pool(name="et", bufs=8) as etp,
        tc.tile_pool(name="sumt", bufs=6) as sumt,
        tc.tile_pool(name="stp", bufs=2, space="PSUM") as stp,
        tc.tile_pool(name="otp", bufs=2, space="PSUM") as otp,
        tc.tile_pool(name="nrm", bufs=2) as nrm,
    ):
        NQH = LQ // QH
        # kt-outer / qh-inner: K^T quarters are consumed at half the rate
        # (DMA keeps up during the ramp) and there is no mid-loop q-half
        # transition. Per-half E^T partial sums on DVE (fp16, 2x mode):
        # pairs -> linear chain of pairs, so the post-loop tail is short.
        ot_list = [otp.tile([128, QH], F32, tag="ot", name=f"ot{i}") for i in range(NQH)]
        pendings = [dict() for _ in range(NQH)]
        chains = [None] * NQH

        def sum_insert(qh, tile_):
            pending = pendings[qh]
            if 0 not in pending:
                pending[0] = tile_
                return
            prev = pending.pop(0)
            pair = sumt.tile([128, QH], FP16, tag="sum0", name="s0")
            nc.vector.tensor_add(out=pair, in0=prev, in1=tile_)
            if chains[qh] is None:
                chains[qh] = pair
            else:
                acc = sumt.tile([128, QH], FP16, tag="sumc", name="sc")
                nc.vector.tensor_add(out=acc, in0=chains[qh], in1=pair)
                chains[qh] = acc

        def iteration(kt, qh):
            st = stp.tile([128, QH], F32, tag="st", name="st")
            for c in range(NCH):
                sl = slice(c * 512, (c + 1) * 512)
                nc.tensor.matmul(
                    st[:, sl],
                    kt_tile(kt),
                    qt_q[qh][:, c * 512 : (c + 1) * 512],
                    start=True,
                    stop=True,
                )
            et = etp.tile([128, QH], FP16, tag="et", name="et")
            nc.scalar.activation(
                out=et, in_=st, func=mybir.ActivationFunctionType.Exp, scale=SCALE
            )
            sum_insert(qh, et)
            for c in range(NCH):
                sl = slice(c * 512, (c + 1) * 512)
                nc.tensor.matmul(
                    ot_list[qh][:, sl],
                    vn_q[kt // 8][:, kt % 8, :],
                    et[:, sl],
                    start=kt == 0,
                    stop=kt == NKT - 1,
                )

        def epilogue(qh):
            q0 = qh * QH
            esum = chains[qh]
            # Partition-reduce esum with an all-ones [128,128] stationary so
            # the result lands replicated across partitions (no broadcast);
            # recip/mul/DMA chunked so the chain pipelines. se borrows an st
            # slot; the normalize multiply reads O^T PSUM directly.
            se_ps = stp.tile([128, QH], F32, tag="st", name="se")
            for c in range(NCH):
                sl = slice(c * 512, (c + 1) * 512)
                nc.tensor.matmul(
                    se_ps[:, sl], ones_mat, esum[:, sl], start=True, stop=True
                )
                rec = nrm.tile([128, 512], F32, tag="rec", name="rec")
                nc.vector.reciprocal(out=rec, in_=se_ps[:, sl])
                nc.vector.tensor_mul(
                    out=otn_h[qh][:, sl], in0=ot_list[qh][:, sl], in1=rec
                )
                nc.sync.dma_start(
                    out=out[:, q0 + c * 512 : q0 + (c + 1) * 512],
                    in_=otn_h[qh][:, sl],
                )

        for i in range(NKT):
            iteration(i, 0)
            iteration(i, 1)
        epilogue(0)
        epilogue(1)


def build_nc() -> "bacc.Bacc":
    nc = bacc.Bacc("TRN2", target_bir_lowering=False, debug=False, num_devices=8)
    io = {}
    io["x1"] = nc.dram_tensor("x1", [LQ, D], F32, kind="ExternalInput").ap()
    io["x2"] = nc.dram_tensor("x2", [LKV, D], F32, kind="ExternalInput").ap()
    io["x3"] = nc.dram_tensor("x3", [LKV, D], F32, kind="ExternalInput").ap()
    io["wpack"] = nc.dram_tensor("wpack", [128, 515], F32, kind="ExternalInput").ap()
    io["o"] = nc.dram_tensor("o", [128, LQ], F32, kind="ExternalOutput").ap()
    with tile.TileContext(nc) as tc:
        attn_body(tc, io)
    nc.compile()
    return nc


def make_in_maps(inputs: dict) -> list[dict]:
    wpack = np.concatenate(
        [
            np.asarray(inputs["Wq"], np.float32),
            np.asarray(inputs["Wk"], np.float32),
            np.asarray(inputs["Wv"], np.float32),
            np.eye(128, dtype=np.float32),
            np.asarray(inputs["bq"], np.float32)[:, None],
            np.asarray(inputs["bk"], np.float32)[:, None],
            np.asarray(inputs["bv"], np.float32)[:, None],
        ],
        axis=1,
    )
    shared = {"wpack": np.ascontiguousarray(wpack)}
    x1 = np.asarray(inputs["x1"], np.float32)
    x2 = np.asarray(inputs["x2"], np.float32)
    x3 = np.asarray(inputs["x3"], np.float32)
    in_maps = []
    for c in range(8):
        b, qh = c // 2, c % 2
        in_maps.append(
            {
                "x1": np.ascontiguousarray(x1[b, qh * LQ : (qh + 1) * LQ, :]),
                "x2": np.ascontiguousarray(x2[b]),
                "x3": np.ascontiguousarray(x3[b]),
                **shared,
            }
        )
    return in_maps


_NC_CACHE = None


def get_nc():
    global _NC_CACHE
    if _NC_CACHE is None:
        _NC_CACHE = build_nc()
    return _NC_CACHE


def kernel(**inputs) -> np.ndarray:
    nc = get_nc()
    in_maps = make_in_maps(inputs)
    res = run_bass_kernel_spmd(nc, in_maps, core_ids=list(range(8)))
    out = np.empty((4, 4096, 128), np.float32)
    for c in range(8):
        b, qh = c // 2, c % 2
        out[b, qh * LQ : (qh + 1) * LQ, :] = res.results[c]["o"].T
    return out


if __name__ == "__main__":
    nc = build_nc()
    print("built OK")

